# revision 1
# baseline (speedup 1.0000x reference)
"""Trainium2 Bass kernel for nn_CrossAttention (B=4, N=2048, C=1024, H=16).

Sharding: 8 cores = 4 batches x 2 query-stripe halves. Core (b, half)
computes all 16 heads for the 1024 query rows in the 128-row blocks
{2j+half : j=0..7} of batch b. Each core computes its batch's full K/V
projection (duplicated across the pair), so there is no cross-core
communication at all; outputs are disjoint row-slices of the final
projection.

Causality: query blocks are processed in pairs (256 rows, global blocks
4p+half and 4p+2+half). Pair p attends to key chunks 0..4p+3; the last
4 key chunks get a multiplicative mask that depends only on `half`, so
a single compiled kernel serves all 8 cores with the mask passed as
data.

Attention math (per head h, pair p):
  S^T[k,q] = sum_d kT[d,k] qhT[d,q]     (64-contraction matmuls; heads
             are processed in even/odd pairs so consecutive matmuls hit
             disjoint PE row groups and run concurrently)
  E = exp(0.125 * S^T)   (no max-subtraction; logits are O(1))
  E tail *= mask
  [Y^T; denom] = matmul(lhsT=[v|1] chunk, rhs=E)  -> psum [65, 256]
  yT = Y^T * broadcast(1/denom)   (gpsimd partition_broadcast)
Then out rows = yT^T @ projT + bias via natural matmuls, with the bias
added as a rank-1 accumulating matmul. yT is written into the storage
of qhT (each 256-column slice of qhT is dead once the same pair's
scores are done, so the attention output can reuse it in place).

Independent projection work (kv/q/out projections) is interleaved into
the attention stream as "background groups" popped between score/AV
steps, so the PE stays busy while ScalarE computes exponentials.
"""

import collections

import numpy as np
import ml_dtypes

import concourse.bass as bass
import concourse.tile as tile
from concourse import bacc, mybir

P = 128
B, N, C, H = 4, 2048, 1024, 16
D = C // H  # 64
NQ = 1024  # query rows per core
IT = C // P  # 8 input-feature tiles
NQT = NQ // P  # 8 query blocks per core
NKT = N // P  # 16 key chunks
NPAIR = 4  # query pairs of 256 rows per core
BF = mybir.dt.bfloat16
F32 = mybir.dt.float32
bf16 = ml_dtypes.bfloat16
EXP = mybir.ActivationFunctionType.Exp
SCALE = float(D) ** -0.5


def _emit_body(nc, pools, dram):
    (consts, acts, xq, wstream, epool, rpool, opool, apsum, spsum, ypsum) = pools
    (qT_d, xT_d, wqT_d, wkT_d, wvT_d, projT_d, bias_d, mask_d, out_d) = dram

    # ---- persistent SBUF tensors ----
    kT_sb = acts.tile([P, IT, N], BF)
    v_sb = acts.tile([P, NKT, H, D + 1], BF)
    qhT_sb = acts.tile([P, IT, NQ], BF)
    yT_sb = qhT_sb  # aliased: each qhT 256-col slice is dead after its pair
    projT_sb = acts.tile([P, IT, C], BF)
    wkT_sb = acts.tile([P, IT, C], BF)
    wvT_sb = acts.tile([P, IT, C], BF)
    mask_sb = consts.tile([P, 4, 256], BF)
    bias_sb = consts.tile([1, C], BF)
    onesb_sb = consts.tile([1, P], BF)

    def load_sliced(dst, dram_ap, engine=None, cols=None):
        """Per-i-tile DMA slices: contiguous DRAM lines + incremental
        availability (consumers depend only on their slice)."""
        eng = engine or nc.sync
        src = dram_ap
        for it in range(IT):
            if cols is None:
                eng.dma_start(dst[:, it], src[:, it])
            else:
                eng.dma_start(dst[:, it], src[:, it, cols])

    # bulk background loads go on the gpsimd (SWDGE) queue so they
    # stream in parallel with the critical-path sync-queue loads
    nc.vector.memset(onesb_sb[:], 1.0)
    nc.vector.memset(v_sb[:, :, :, D : D + 1], 1.0)
    nc.gpsimd.dma_start(wvT_sb[:], wvT_d.rearrange("(i p) o -> p i o", p=P))
    nc.gpsimd.dma_start(mask_sb[:], mask_d.rearrange("(c p) q -> p c q", p=P))
    nc.gpsimd.dma_start(bias_sb[:], bias_d[None, :])
    nc.gpsimd.dma_start(projT_sb[:], projT_d.rearrange("(c p) o -> p c o", p=P))

    # -------- background projection groups --------
    bg = collections.deque()

    def qh_batch(nh):
        qt = xq.tile([P, IT, 512], BF, tag="xq")
        wt = wstream.tile([P, IT, C], BF, tag="wq")
        qsrc = qT_d.rearrange("(i p) n -> p i n", p=P)
        wsrc = wqT_d.rearrange("(i p) o -> p i o", p=P)
        for it in range(IT):  # interleaved so it-slice deps land in order
            nc.sync.dma_start(wt[:, it], wsrc[:, it])
            nc.sync.dma_start(qt[:, it], qsrc[:, it, nh * 512 : (nh + 1) * 512])

        def group(ot):
            def emit():
                ps = apsum.tile([P, 512], F32, tag="acc")
                for it in range(IT):
                    nc.tensor.matmul(
                        ps[:],
                        lhsT=wt[:, it, ot * P : (ot + 1) * P],
                        rhs=qt[:, it, :],
                        start=(it == 0),
                        stop=(it == IT - 1),
                    )
                nc.vector.tensor_copy(qhT_sb[:, ot, nh * 512 : (nh + 1) * 512], ps[:])

            return emit

        return [group(ot) for ot in range(IT)]

    def kv_batch(nh):
        xt = xq.tile([P, IT, 512], BF, tag="xq")
        load_sliced(
            xt, xT_d.rearrange("(i p) n -> p i n", p=P),
            cols=slice(nh * 512, (nh + 1) * 512),
        )

        def kgroup(ot):
            def emit():
                ps = apsum.tile([P, 512], F32, tag="acc")
                for it in range(IT):
                    nc.tensor.matmul(
                        ps[:],
                        lhsT=wkT_sb[:, it, ot * P : (ot + 1) * P],
                        rhs=xt[:, it, :],
                        start=(it == 0),
                        stop=(it == IT - 1),
                    )
                nc.vector.tensor_copy(kT_sb[:, ot, nh * 512 : (nh + 1) * 512], ps[:])

            return emit

        def vgroup(oh, ntl):
            def emit():
                nt = nh * 4 + ntl
                ps = apsum.tile([P, 512], F32, tag="acc")
                for it in range(IT):
                    nc.tensor.matmul(
                        ps[:],
                        lhsT=xt[:, it, ntl * P : (ntl + 1) * P],
                        rhs=wvT_sb[:, it, oh * 512 : (oh + 1) * 512],
                        start=(it == 0),
                        stop=(it == IT - 1),
                    )
                nc.vector.tensor_copy(
                    v_sb[:, nt, oh * 8 : (oh + 1) * 8, 0:D],
                    ps[:].rearrange("p (h d) -> p h d", d=D),
                )

            return emit

        groups = [kgroup(ot) for ot in range(IT)]
        groups += [vgroup(oh, ntl) for oh in range(2) for ntl in range(4)]
        return groups

    def proj_batch(pp):
        def pgroup(nt, oh):
            def emit():
                ps = apsum.tile([P, 512], F32, tag="acc")
                for ct in range(IT):
                    nc.tensor.matmul(
                        ps[:],
                        lhsT=yT_sb[:, ct, nt * P : (nt + 1) * P],
                        rhs=projT_sb[:, ct, oh * 512 : (oh + 1) * 512],
                        start=(ct == 0),
                        stop=False,
                    )
                nc.tensor.matmul(
                    ps[:],
                    lhsT=onesb_sb[0:1, :],
                    rhs=bias_sb[0:1, oh * 512 : (oh + 1) * 512],
                    start=False,
                    stop=True,
                )
                ot_sb = opool.tile([P, 512], F32, tag="o")
                nc.vector.tensor_copy(ot_sb[:], ps[:])
                nc.sync.dma_start(
                    out_d[nt * P : (nt + 1) * P, oh * 512 : (oh + 1) * 512], ot_sb[:]
                )

            return emit

        return [pgroup(nt, oh) for nt in (2 * pp, 2 * pp + 1) for oh in range(2)]

    total_steps = sum(8 * (p + 2) for p in range(NPAIR))
    state = {"steps_left": total_steps, "credit": 0.0}

    def pop_bg():
        # spread background groups evenly over the remaining steps so
        # the PE has filler work during the ACT-bound attention tail
        state["credit"] += len(bg) / max(1, state["steps_left"])
        while state["credit"] >= 1.0 and bg:
            bg.popleft()[1]()
            state["credit"] -= 1.0
        state["steps_left"] -= 1

    def flush_bg(pair):
        """Emit every queued group some unit of `pair` will read.
        Tile resolves dependencies from emission history only, so a
        writer must always be emitted before its readers."""
        while any(d <= pair for d, _ in bg):
            bg.popleft()[1]()

    # -------- upfront projections (needed by pair 0) --------
    qh0 = qh_batch(0)  # emits qt + wq DMAs first on the sync queue
    load_sliced(wkT_sb, wkT_d.rearrange("(i p) o -> p i o", p=P))
    kv0 = kv_batch(0)
    for g_ in qh0:
        g_()
    for g_ in kv0:
        g_()

    # -------- attention with interleaved background --------
    for p in range(NPAIR):
        if p == 0:
            # qh columns 512.. are first read by pair 2
            bg.extend((2, g) for g in qh_batch(1))
        if p < NPAIR - 1:
            bg.extend((p + 1, g) for g in kv_batch(p + 1))
        if p >= 1:
            bg.extend((NPAIR + 1, g) for g in proj_batch(p - 1))
        flush_bg(p)
        qs = slice(p * 256, (p + 1) * 256)
        for m in range(8):
            h0, h1 = 2 * m, 2 * m + 1
            yps0 = ypsum.tile([P, 256], F32, tag="y")
            yps1 = ypsum.tile([P, 256], F32, tag="y")

            def do_av(ework, last):
                e0, e1, g = ework
                for et, yps, h in ((e0, yps0, h0), (e1, yps1, h1)):
                    for cc in range(4):
                        c = g * 4 + cc
                        nc.tensor.matmul(
                            yps[0 : D + 1, :],
                            lhsT=v_sb[:, c, h, :],
                            rhs=et[:, cc, :],
                            start=(g == 0 and cc == 0),
                            stop=(last and cc == 3),
                        )

            prev = None  # (E0, E1, g)
            for g in range(p + 1):
                pop_bg()
                es = []
                for hb in (0, D):
                    sps = spsum.tile([P, 1024], F32, tag="s")
                    for cc in range(4):
                        c = g * 4 + cc
                        nc.tensor.matmul(
                            sps[:, cc * 256 : (cc + 1) * 256],
                            lhsT=kT_sb[hb : hb + D, m, c * P : (c + 1) * P],
                            rhs=qhT_sb[hb : hb + D, m, qs],
                            start=True,
                            stop=True,
                        )
                    et = epool.tile([P, 4, 256], BF, tag="e")
                    nc.scalar.activation(
                        et[:].rearrange("p a b -> p (a b)"), sps[:], EXP, scale=SCALE
                    )
                    if g == p:
                        nc.vector.tensor_mul(et[:], et[:], mask_sb[:])
                    es.append(et)
                if prev is not None:
                    do_av(prev, last=False)
                prev = (es[0], es[1], g)
            pop_bg()
            do_av(prev, last=True)
            # normalize both heads
            for yps, hb in ((yps0, 0), (yps1, D)):
                r = rpool.tile([1, 256], F32, tag="r")
                nc.vector.reciprocal(r[:], yps[D : D + 1, :])
                rsb = rpool.tile([D, 256], F32, tag="rb")
                nc.gpsimd.partition_broadcast(rsb[:], r[:])
                nc.vector.tensor_mul(yT_sb[hb : hb + D, m, qs], yps[0:D, :], rsb[:])

    bg.extend((NPAIR + 1, g) for g in proj_batch(NPAIR - 1))
    while bg:
        bg.popleft()[1]()


def build_nc(loop_iters=None):
    nc = bacc.Bacc("TRN2", target_bir_lowering=False, debug=False, num_devices=8)

    dram = (
        nc.dram_tensor("qT", [C, NQ], BF, kind="ExternalInput").ap(),
        nc.dram_tensor("xT", [C, N], BF, kind="ExternalInput").ap(),
        nc.dram_tensor("wqT", [C, C], BF, kind="ExternalInput").ap(),
        nc.dram_tensor("wkT", [C, C], BF, kind="ExternalInput").ap(),
        nc.dram_tensor("wvT", [C, C], BF, kind="ExternalInput").ap(),
        nc.dram_tensor("projT", [C, C], BF, kind="ExternalInput").ap(),
        nc.dram_tensor("bias", [C], BF, kind="ExternalInput").ap(),
        nc.dram_tensor("mask", [4 * P, 256], BF, kind="ExternalInput").ap(),
        nc.dram_tensor("out", [NQ, C], F32, kind="ExternalOutput").ap(),
    )

    with tile.TileContext(nc) as tc:
        with (
            tc.tile_pool(name="consts", bufs=1) as consts,
            tc.tile_pool(name="acts", bufs=1) as acts,
            tc.tile_pool(name="xq", bufs=2) as xq,
            tc.tile_pool(name="wstream", bufs=1) as wstream,
            tc.tile_pool(name="epool", bufs=3) as epool,
            tc.tile_pool(name="rpool", bufs=2) as rpool,
            tc.tile_pool(name="opool", bufs=2) as opool,
            tc.tile_pool(name="apsum", bufs=2, space="PSUM") as apsum,
            tc.tile_pool(name="spsum", bufs=2, space="PSUM") as spsum,
            tc.tile_pool(name="ypsum", bufs=2, space="PSUM") as ypsum,
        ):
            pools = (
                consts, acts, xq, wstream, epool, rpool, opool,
                apsum, spsum, ypsum,
            )
            if loop_iters:
                with tc.For_i(0, loop_iters, 1):
                    _emit_body(nc, pools, dram)
            else:
                _emit_body(nc, pools, dram)

    nc.compile()
    return nc


def make_mask(half):
    """Multiplicative causal mask for the 4 tail key chunks vs the 2
    query blocks of a pair: mask[rk, b*128+qq] = rk <= (2b+half)*128+qq."""
    rk = np.arange(4 * P)[:, None]
    b = np.arange(256)[None, :] // P
    qq = np.arange(256)[None, :] % P
    return (rk <= (2 * b + half) * P + qq).astype(bf16)


def prep_inputs(q, x, wq_w, wkv_w, proj_w, proj_b):
    wqT = np.ascontiguousarray(wq_w.T).astype(bf16)
    wkT = np.ascontiguousarray(wkv_w[:C].T).astype(bf16)
    wvT = np.ascontiguousarray(wkv_w[C:].T).astype(bf16)
    projT = np.ascontiguousarray(proj_w.T).astype(bf16)
    bias = proj_b.astype(bf16)
    masks = [make_mask(0), make_mask(1)]
    in_maps = []
    for core in range(8):
        b, half = core // 2, core % 2
        blocks = [2 * j + half for j in range(NQT)]
        qrows = q[b].reshape(NKT, P, C)[blocks].reshape(NQ, C)
        in_maps.append(
            dict(
                qT=np.ascontiguousarray(qrows.T).astype(bf16),
                xT=np.ascontiguousarray(x[b].T).astype(bf16),
                wqT=wqT,
                wkT=wkT,
                wvT=wvT,
                projT=projT,
                bias=bias,
                mask=masks[half],
            )
        )
    return in_maps


def assemble_output(results):
    out = np.empty((B, N, C), np.float32)
    for core in range(8):
        b, half = core // 2, core % 2
        blocks = [2 * j + half for j in range(NQT)]
        out[b].reshape(NKT, P, C)[blocks] = results[core]["out"].reshape(NQT, P, C)
    return out


_CACHE = {}


def kernel(q, x, wq_w, wkv_w, proj_w, proj_b):
    in_maps = prep_inputs(
        np.asarray(q), np.asarray(x), np.asarray(wq_w), np.asarray(wkv_w),
        np.asarray(proj_w), np.asarray(proj_b),
    )
    if "nc" not in _CACHE:
        _CACHE["nc"] = build_nc()
    nc = _CACHE["nc"]
    from concourse.bass_utils import run_bass_kernel_spmd

    res = run_bass_kernel_spmd(nc, in_maps, list(range(8)))
    return assemble_output(res.results)


if __name__ == "__main__":
    rng = np.random.default_rng(0)
    ins = {
        "q": rng.standard_normal((B, N, C)).astype(np.float32),
        "x": rng.standard_normal((B, N, C)).astype(np.float32),
        "wq_w": (rng.standard_normal((C, C)) * 0.02).astype(np.float32),
        "wkv_w": (rng.standard_normal((2 * C, C)) * 0.02).astype(np.float32),
        "proj_w": (rng.standard_normal((C, C)) * 0.02).astype(np.float32),
        "proj_b": np.zeros(C, np.float32),
    }
    out = kernel(**ins)
    print(out.shape, out.dtype)



# revision 17
# speedup vs baseline: 1.1761x; 1.1761x over previous
"""Trainium2 Bass kernel for nn_CrossAttention (B=4, N=2048, C=1024, H=16).

Sharding: 8 cores = 4 batches x 2 query-stripe halves. Core (b, half)
computes all 16 heads for the 1024 query rows in the 128-row blocks
{2j+half : j=0..7} of batch b. Each core computes its batch's full K/V
projection (duplicated across the pair), so there is no cross-core
communication at all; outputs are disjoint row-slices of the final
projection.

Causality: query blocks are processed in pairs (256 rows, global blocks
4p+half and 4p+2+half). Pair p attends to key chunks 0..4p+3; the last
4 key chunks get a multiplicative mask that depends only on `half`, so
a single compiled kernel serves all 8 cores with the mask passed as
data.

Attention math (per head h, pair p):
  S^T[k,q] = sum_d kT[d,k] qhT[d,q]     (64-contraction matmuls; heads
             are processed in even/odd pairs so consecutive matmuls hit
             disjoint PE row groups and run concurrently)
  E = exp(0.125 * S^T)   (no max-subtraction; logits are O(1))
  E tail *= mask
  [Y^T; denom] = matmul(lhsT=[v|1] chunk, rhs=E)  -> psum [65, 256]
  yT = Y^T * broadcast(1/denom)   (gpsimd partition_broadcast)
Then out rows = yT^T @ projT + bias via natural matmuls, with the bias
added as a rank-1 accumulating matmul. yT is written into the storage
of qhT (each 256-column slice of qhT is dead once the same pair's
scores are done, so the attention output can reuse it in place).

Independent projection work (kv/q/out projections) is interleaved into
the attention stream as "background groups" popped between score/AV
steps, so the PE stays busy while ScalarE computes exponentials.
"""

import collections

import numpy as np
import ml_dtypes

import concourse.bass as bass
import concourse.tile as tile
from concourse import bacc, mybir

P = 128
B, N, C, H = 4, 2048, 1024, 16
D = C // H  # 64
NQ = 1024  # query rows per core
IT = C // P  # 8 input-feature tiles
NQT = NQ // P  # 8 query blocks per core
NKT = N // P  # 16 key chunks
NPAIR = 4  # query pairs of 256 rows per core
BF = mybir.dt.bfloat16
F32 = mybir.dt.float32
bf16 = ml_dtypes.bfloat16
EXP = mybir.ActivationFunctionType.Exp
SCALE = float(D) ** -0.5


def _emit_body(nc, pools, dram):
    (consts, acts, xq, wstream, epool, rpool, opool, apsum, spsum, ypsum) = pools
    (qT_d, xT_d, wqT_d, wkT_d, wvT_d, projT_d, bias_d, mask_d, out_d) = dram

    # ---- persistent SBUF tensors ----
    kT_sb = acts.tile([P, IT, N], BF)
    v_sb = acts.tile([P, NKT, H, D + 1], BF)
    qhT_sb = acts.tile([P, IT, NQ], BF)
    yT_sb = qhT_sb  # aliased: each qhT 256-col slice is dead after its pair
    projT_sb = acts.tile([P, IT, C], BF)
    wkT_sb = acts.tile([P, IT, C], BF)
    wvT_sb = acts.tile([P, IT, C], BF)
    mask_sb = consts.tile([P, 4, 256], BF)
    bias_sb = consts.tile([1, C], BF)
    onesb_sb = consts.tile([1, P], BF)

    def load_sliced(dst, dram_ap, engine=None, cols=None):
        """Per-i-tile DMA slices: contiguous DRAM lines + incremental
        availability (consumers depend only on their slice)."""
        eng = engine or nc.sync
        src = dram_ap
        for it in range(IT):
            if cols is None:
                eng.dma_start(dst[:, it], src[:, it])
            else:
                eng.dma_start(dst[:, it], src[:, it, cols])

    # bulk background loads go on the gpsimd (SWDGE) queue so they
    # stream in parallel with the critical-path sync-queue loads
    nc.vector.memset(onesb_sb[:], 1.0)
    nc.vector.memset(v_sb[:, :, :, D : D + 1], 1.0)
    nc.gpsimd.dma_start(wvT_sb[:], wvT_d.rearrange("(i p) o -> p i o", p=P))
    nc.gpsimd.dma_start(mask_sb[:], mask_d.rearrange("(c p) q -> p c q", p=P))
    nc.gpsimd.dma_start(bias_sb[:], bias_d[None, :])
    nc.gpsimd.dma_start(projT_sb[:], projT_d.rearrange("(c p) o -> p c o", p=P))

    # -------- background projection groups --------
    bg = collections.deque()

    def qh_batch(nh):
        qt = xq.tile([P, IT, 512], BF, tag="xq")
        wt = wstream.tile([P, IT, C], BF, tag="wq")
        qsrc = qT_d.rearrange("(i p) n -> p i n", p=P)
        wsrc = wqT_d.rearrange("(i p) o -> p i o", p=P)
        for it in range(IT):  # interleaved so it-slice deps land in order
            nc.sync.dma_start(wt[:, it], wsrc[:, it])
            nc.sync.dma_start(qt[:, it], qsrc[:, it, nh * 512 : (nh + 1) * 512])

        def group(ot):
            def emit():
                ps = apsum.tile([P, 512], F32, tag="acc")
                for it in range(IT):
                    nc.tensor.matmul(
                        ps[:],
                        lhsT=wt[:, it, ot * P : (ot + 1) * P],
                        rhs=qt[:, it, :],
                        start=(it == 0),
                        stop=(it == IT - 1),
                    )
                nc.vector.tensor_copy(qhT_sb[:, ot, nh * 512 : (nh + 1) * 512], ps[:])

            return emit

        return [group(ot) for ot in range(IT)]

    def kv_batch(nh):
        xt = xq.tile([P, IT, 512], BF, tag="xq")
        load_sliced(
            xt, xT_d.rearrange("(i p) n -> p i n", p=P),
            cols=slice(nh * 512, (nh + 1) * 512),
        )

        def kgroup(ot):
            def emit():
                ps = apsum.tile([P, 512], F32, tag="acc")
                for it in range(IT):
                    nc.tensor.matmul(
                        ps[:],
                        lhsT=wkT_sb[:, it, ot * P : (ot + 1) * P],
                        rhs=xt[:, it, :],
                        start=(it == 0),
                        stop=(it == IT - 1),
                    )
                nc.vector.tensor_copy(kT_sb[:, ot, nh * 512 : (nh + 1) * 512], ps[:])

            return emit

        def vgroup(oh, ntl):
            def emit():
                nt = nh * 4 + ntl
                ps = apsum.tile([P, 512], F32, tag="acc")
                for it in range(IT):
                    nc.tensor.matmul(
                        ps[:],
                        lhsT=xt[:, it, ntl * P : (ntl + 1) * P],
                        rhs=wvT_sb[:, it, oh * 512 : (oh + 1) * 512],
                        start=(it == 0),
                        stop=(it == IT - 1),
                    )
                nc.vector.tensor_copy(
                    v_sb[:, nt, oh * 8 : (oh + 1) * 8, 0:D],
                    ps[:].rearrange("p (h d) -> p h d", d=D),
                )

            return emit

        groups = [kgroup(ot) for ot in range(IT)]
        groups += [vgroup(oh, ntl) for oh in range(2) for ntl in range(4)]
        return groups

    def proj_batch(pp):
        def pgroup(nt, oh):
            def emit():
                ps = apsum.tile([P, 512], F32, tag="acc")
                for ct in range(IT):
                    nc.tensor.matmul(
                        ps[:],
                        lhsT=yT_sb[:, ct, nt * P : (nt + 1) * P],
                        rhs=projT_sb[:, ct, oh * 512 : (oh + 1) * 512],
                        start=(ct == 0),
                        stop=False,
                    )
                nc.tensor.matmul(
                    ps[:],
                    lhsT=onesb_sb[0:1, :],
                    rhs=bias_sb[0:1, oh * 512 : (oh + 1) * 512],
                    start=False,
                    stop=True,
                )
                ot_sb = opool.tile([P, 512], F32, tag="o")
                nc.vector.tensor_copy(ot_sb[:], ps[:])
                nc.sync.dma_start(
                    out_d[nt * P : (nt + 1) * P, oh * 512 : (oh + 1) * 512], ot_sb[:]
                )

            return emit

        return [pgroup(nt, oh) for nt in (2 * pp, 2 * pp + 1) for oh in range(2)]

    total_steps = sum(8 * (p + 2) for p in range(NPAIR))
    state = {"steps_left": total_steps, "credit": 0.0}

    def pop_bg():
        # spread background groups evenly over the remaining steps so
        # the PE has filler work during the ACT-bound attention tail
        state["credit"] += len(bg) / max(1, state["steps_left"])
        while state["credit"] >= 1.0 and bg:
            bg.popleft()[1]()
            state["credit"] -= 1.0
        state["steps_left"] -= 1

    def flush_bg(pair):
        """Emit every queued group some unit of `pair` will read.
        Tile resolves dependencies from emission history only, so a
        writer must always be emitted before its readers."""
        while any(d <= pair for d, _ in bg):
            bg.popleft()[1]()

    # -------- upfront projections (needed by pair 0) --------
    qh0 = qh_batch(0)  # emits qt + wq DMAs first on the sync queue
    load_sliced(wkT_sb, wkT_d.rearrange("(i p) o -> p i o", p=P))
    kv0 = kv_batch(0)
    for g_ in qh0:
        g_()
    for g_ in kv0:
        g_()

    # -------- attention with interleaved background --------
    for p in range(NPAIR):
        if p == 0:
            # qh columns 512.. are first read by pair 2
            bg.extend((2, g) for g in qh_batch(1))
        if p < NPAIR - 1:
            bg.extend((p + 1, g) for g in kv_batch(p + 1))
        if p >= 1:
            bg.extend((NPAIR + 1, g) for g in proj_batch(p - 1))
        flush_bg(p)
        qs = slice(p * 256, (p + 1) * 256)
        for m in range(8):
            h0, h1 = 2 * m, 2 * m + 1
            yps0 = ypsum.tile([P, 256], F32, tag="y")
            yps1 = ypsum.tile([P, 256], F32, tag="y")

            def do_av(ework, last):
                e0, e1, g = ework
                for et, yps, h in ((e0, yps0, h0), (e1, yps1, h1)):
                    for cc in range(4):
                        c = g * 4 + cc
                        nc.tensor.matmul(
                            yps[0 : D + 1, :],
                            lhsT=v_sb[:, c, h, :],
                            rhs=et[:, cc, :],
                            start=(g == 0 and cc == 0),
                            stop=(last and cc == 3),
                        )

            prev = None  # (E0, E1, g)
            for g in range(p + 1):
                pop_bg()
                es = []
                for hb in (0, D):
                    sps = spsum.tile([P, 1024], F32, tag="s")
                    for cc in range(4):
                        c = g * 4 + cc
                        nc.tensor.matmul(
                            sps[:, cc * 256 : (cc + 1) * 256],
                            lhsT=kT_sb[hb : hb + D, m, c * P : (c + 1) * P],
                            rhs=qhT_sb[hb : hb + D, m, qs],
                            start=True,
                            stop=True,
                        )
                    et = epool.tile([P, 4, 256], BF, tag="e")
                    nc.scalar.activation(
                        et[:].rearrange("p a b -> p (a b)"), sps[:], EXP, scale=SCALE
                    )
                    if g == p:
                        nc.vector.tensor_mul(et[:], et[:], mask_sb[:])
                    es.append(et)
                if prev is not None:
                    do_av(prev, last=False)
                prev = (es[0], es[1], g)
            pop_bg()
            do_av(prev, last=True)
            # normalize both heads
            for yps, hb in ((yps0, 0), (yps1, D)):
                r = rpool.tile([1, 256], F32, tag="r")
                nc.vector.reciprocal(r[:], yps[D : D + 1, :])
                rsb = rpool.tile([D, 256], F32, tag="rb")
                nc.gpsimd.partition_broadcast(rsb[:], r[:])
                nc.vector.tensor_mul(yT_sb[hb : hb + D, m, qs], yps[0:D, :], rsb[:])

    bg.extend((NPAIR + 1, g) for g in proj_batch(NPAIR - 1))
    while bg:
        bg.popleft()[1]()


def build_nc(loop_iters=None):
    nc = bacc.Bacc("TRN2", target_bir_lowering=False, debug=False, num_devices=8)

    dram = (
        nc.dram_tensor("qT", [C, NQ], BF, kind="ExternalInput").ap(),
        nc.dram_tensor("xT", [C, N], BF, kind="ExternalInput").ap(),
        nc.dram_tensor("wqT", [C, C], BF, kind="ExternalInput").ap(),
        nc.dram_tensor("wkT", [C, C], BF, kind="ExternalInput").ap(),
        nc.dram_tensor("wvT", [C, C], BF, kind="ExternalInput").ap(),
        nc.dram_tensor("projT", [C, C], BF, kind="ExternalInput").ap(),
        nc.dram_tensor("bias", [C], BF, kind="ExternalInput").ap(),
        nc.dram_tensor("mask", [4 * P, 256], BF, kind="ExternalInput").ap(),
        nc.dram_tensor("out", [NQ, C], F32, kind="ExternalOutput").ap(),
    )

    with tile.TileContext(nc) as tc:
        with (
            tc.tile_pool(name="consts", bufs=1) as consts,
            tc.tile_pool(name="acts", bufs=1) as acts,
            tc.tile_pool(name="xq", bufs=2) as xq,
            tc.tile_pool(name="wstream", bufs=1) as wstream,
            tc.tile_pool(name="epool", bufs=3) as epool,
            tc.tile_pool(name="rpool", bufs=2) as rpool,
            tc.tile_pool(name="opool", bufs=2) as opool,
            tc.tile_pool(name="apsum", bufs=2, space="PSUM") as apsum,
            tc.tile_pool(name="spsum", bufs=2, space="PSUM") as spsum,
            tc.tile_pool(name="ypsum", bufs=2, space="PSUM") as ypsum,
        ):
            pools = (
                consts, acts, xq, wstream, epool, rpool, opool,
                apsum, spsum, ypsum,
            )
            if loop_iters:
                with tc.For_i(0, loop_iters, 1):
                    _emit_body(nc, pools, dram)
            else:
                _emit_body(nc, pools, dram)

    nc.compile()
    return nc


def make_mask(half):
    """Multiplicative causal mask for the 4 tail key chunks vs the 2
    query blocks of a pair: mask[rk, b*128+qq] = rk <= (2b+half)*128+qq."""
    rk = np.arange(4 * P)[:, None]
    b = np.arange(256)[None, :] // P
    qq = np.arange(256)[None, :] % P
    return (rk <= (2 * b + half) * P + qq).astype(bf16)


def prep_inputs(q, x, wq_w, wkv_w, proj_w, proj_b):
    wqT = np.ascontiguousarray(wq_w.T).astype(bf16)
    wkT = np.ascontiguousarray(wkv_w[:C].T).astype(bf16)
    wvT = np.ascontiguousarray(wkv_w[C:].T).astype(bf16)
    projT = np.ascontiguousarray(proj_w.T).astype(bf16)
    bias = proj_b.astype(bf16)
    masks = [make_mask(0), make_mask(1)]
    in_maps = []
    for core in range(8):
        b, half = core // 2, core % 2
        blocks = [2 * j + half for j in range(NQT)]
        qrows = q[b].reshape(NKT, P, C)[blocks].reshape(NQ, C)
        in_maps.append(
            dict(
                qT=np.ascontiguousarray(qrows.T).astype(bf16),
                xT=np.ascontiguousarray(x[b].T).astype(bf16),
                wqT=wqT,
                wkT=wkT,
                wvT=wvT,
                projT=projT,
                bias=bias,
                mask=masks[half],
            )
        )
    return in_maps


def assemble_output(results):
    out = np.empty((B, N, C), np.float32)
    for core in range(8):
        b, half = core // 2, core % 2
        blocks = [2 * j + half for j in range(NQT)]
        out[b].reshape(NKT, P, C)[blocks] = results[core]["out"].reshape(NQT, P, C)
    return out


_CACHE = {}


def kernel(q, x, wq_w, wkv_w, proj_w, proj_b):
    in_maps = prep_inputs(
        np.asarray(q), np.asarray(x), np.asarray(wq_w), np.asarray(wkv_w),
        np.asarray(proj_w), np.asarray(proj_b),
    )
    if "nc" not in _CACHE:
        _CACHE["nc"] = build_nc()
    nc = _CACHE["nc"]
    from concourse.bass_utils import run_bass_kernel_spmd

    res = run_bass_kernel_spmd(nc, in_maps, list(range(8)))
    return assemble_output(res.results)


if __name__ == "__main__":
    rng = np.random.default_rng(0)
    ins = {
        "q": rng.standard_normal((B, N, C)).astype(np.float32),
        "x": rng.standard_normal((B, N, C)).astype(np.float32),
        "wq_w": (rng.standard_normal((C, C)) * 0.02).astype(np.float32),
        "wkv_w": (rng.standard_normal((2 * C, C)) * 0.02).astype(np.float32),
        "proj_w": (rng.standard_normal((C, C)) * 0.02).astype(np.float32),
        "proj_b": np.zeros(C, np.float32),
    }
    out = kernel(**ins)
    print(out.shape, out.dtype)

